# revision 48
# baseline (speedup 1.0000x reference)
"""Fused causal MHA kernel for TRN2, one core = (batch b, head-group g of 8 heads).

Layouts (per core):
  xt_all [128, 8, N]  X[b]^T k-blocks stacked  (k%128 on partitions)
  wv_all [128, 8, 512] column shard            (k%128 on partitions)
  wqt/wkt per hp: [128, 8, 128]
  wo_all [128, 4, 1024] row shard
  tri16 [128, 128] fp16 0/1 lower-triangle keep-mask: tri16[j, i] = (i >= j)
  outt [1024, N] fp16 partial (X attn Wo_g)^T; host sums the two
       head-group partials per batch and transposes.

On-chip:
  qt/kt per head-pair hp: [128, N]; partitions = (h0 d0-63, h1 d0-63).
  v per seq m-block: [128, 8*65]; seq on partitions, 8 heads * (64+ones) on free.
  S^T per (hp, c, jb): psum [128, 1024] = h0|h1; j on partitions, i on free.
  Causal masking is post-exp: pt *= tri16 on the diagonal 128-blocks (fp16 DVE),
  instead of adding -1e9 into PSUM pre-exp (slow fp32 PSUM DVE ops).
  PV col-packed: psumO[0:64] = h0 O^T, [64:128] = h1 O^T; 65th V column of
  ones gives the softmax denominator for free.
"""

import os

# recover cleanly if a previous run left the NeuronCores in a degraded
# power/clock state (observed ~19% slowdowns without this)
os.environ.setdefault("NEURON_RT_RESET_CORES", "1")

import numpy as np
import concourse.bass as bass
import concourse.tile as tile
from concourse import bacc, mybir

F32 = mybir.dt.float32
F16 = mybir.dt.float16
F32R = mybir.dt.float32r
AF = mybir.ActivationFunctionType

P = 128
D = 1024
DH = 512  # head-group width: 8 heads * 64
DK = 64
KB = D // P  # 8 k-blocks
NHP = 4  # head-pairs per core
WARM_MM = 24  # PE warm-up matmuls during the DMA lead-in


def build(N=2048, interleave=True):
    MB = N // P  # seq 128-blocks
    MC = N // 512  # seq 512-chunks
    nc = bacc.Bacc("TRN2", target_bir_lowering=False, debug=False)

    xt_d = nc.dram_tensor("xt", [D, N], F16, kind="ExternalInput")
    # host-pretransposed [hp, p, k, m] so each head-pair's block is contiguous
    wq_d = nc.dram_tensor("wq", [NHP, P, KB, P], F16, kind="ExternalInput")
    wk_d = nc.dram_tensor("wk", [NHP, P, KB, P], F16, kind="ExternalInput")
    wv_d = nc.dram_tensor("wv", [D, DH], F16, kind="ExternalInput")
    wo_d = nc.dram_tensor("wo", [DH, D], F16, kind="ExternalInput")
    tri_d = nc.dram_tensor("tri16", [P, P], F16, kind="ExternalInput")
    ones_d = nc.dram_tensor("ones16", [P, DK], F16, kind="ExternalInput")
    out_d = nc.dram_tensor("outt", [D, N], F16, kind="ExternalOutput")

    with tile.TileContext(nc) as tc:
        with (
            tc.tile_pool(name="sb", bufs=1) as sb,
            tc.tile_pool(name="ps", bufs=1, space="PSUM") as ps,
        ):
            # ---- persistent tiles ----
            ones = sb.tile([P, DK], F16, tag="ones", bufs=1)
            tri16 = sb.tile([P, P], F16, tag="tri", bufs=1)
            wv = sb.tile([P, KB, DH], F16, tag="wv", bufs=1, name="wv_all")
            xt = sb.tile([P, KB, N], F16, tag="xt", bufs=1, name="xt_all")
            v = [sb.tile([P, 8 * 65], F16, tag="v", bufs=MB, name=f"v{m}") for m in range(MB)]
            ot = [sb.tile([P, N], F16, tag="ot", bufs=NHP, name=f"ot{t}") for t in range(NHP)]

            nc.sync.dma_start(ones[:], ones_d.ap())
            nc.sync.dma_start(tri16[:], tri_d.ap())
            wv_src = wv_d.ap().rearrange("(k p) n -> p k n", p=P)
            xt_src = xt_d.ap().rearrange("(k p) n -> p k n", p=P)
            # k-split so the first v_proj matmuls can start on half the data
            nc.sync.dma_start(wv[:, 0:4, :], wv_src[:, 0:4, :])
            nc.sync.dma_start(xt[:, 0:4, 0:512], xt_src[:, 0:4, 0:512])
            nc.sync.dma_start(wv[:, 4:6, :], wv_src[:, 4:6, :])
            nc.sync.dma_start(xt[:, 4:6, 0:512], xt_src[:, 4:6, 0:512])
            nc.sync.dma_start(wv[:, 6:KB, :], wv_src[:, 6:KB, :])
            nc.sync.dma_start(xt[:, 6:KB, 0:512], xt_src[:, 6:KB, 0:512])

            def xt_stream(cc):
                nc.sync.dma_start(
                    xt[:, :, cc * 512:(cc + 1) * 512],
                    xt_src[:, :, cc * 512:(cc + 1) * 512],
                )

            # warm the PE clock gate + the ACT exp table during the DMA lead-in
            warm = sb.tile([P, DK], F16, tag="warm", bufs=1, name="warm")
            nc.scalar.activation(warm[:], ones[:], AF.Exp)
            ones32 = sb.tile([P, DK], F32, tag="ones32", bufs=1)
            nc.gpsimd.memset(ones32[:], 1.0)
            psW = ps.tile([P, 512], F32, tag="proj", bufs=2, name="psW")
            for i in range(WARM_MM):
                nc.tensor.matmul(
                    psW[0:64, 0:128], tri16[:, 0:64], tri16[:],
                    start=(i == 0), stop=(i == WARM_MM - 1),
                )

            wo_t = sb.tile([P, NHP, D], F16, tag="wo", bufs=1, name="wo_all")

            # ---- deferred projection work (pumped between attention units) ----
            deferred = []
            dve_deferred = []
            credit = [0.0]
            hold = [0]

            def pump(rate):
                if dve_deferred:
                    dve_deferred.pop(0)()
                credit[0] += rate
                while credit[0] >= 1.0 and len(deferred) > hold[0]:
                    deferred.pop(0)()
                    credit[0] -= 1.0
                if len(deferred) <= hold[0]:
                    credit[0] = 0.0

            def v_proj_k(m, k0, k1, cell):
                if k0 == 0:
                    cell[m] = ps.tile([P, 512], F32, tag="proj", bufs=2, name="psV")
                psV = cell[m]
                for k in range(k0, k1):
                    nc.tensor.matmul(
                        psV[:],
                        xt[:, k, m * P:(m + 1) * P],
                        wv[:, k, :],
                        start=(k == 0),
                        stop=(k == KB - 1),
                    )
                if k1 == KB:
                    v3 = v[m][:].rearrange("p (h x) -> p h x", x=65)
                    nc.vector.tensor_copy(
                        v3[:, :, 0:64], psV[:].rearrange("p (h x) -> p h x", x=64)
                    )
                    nc.vector.tensor_copy(v3[:, :, 64:65], ones[:, 0:8, None])

            def v_proj(m):
                cell = {}
                v_proj_k(m, 0, KB, cell)

            def qk_proj_parts(hp, c, w_all, dst, scale):
                cell = {}

                def part(k0, k1, fin):
                    if k0 == 0:
                        cell["ps"] = ps.tile(
                            [P, 512], F32, tag="proj", bufs=2, name="psQ"
                        )
                    psQ = cell["ps"]
                    for k in range(k0, k1):
                        nc.tensor.matmul(
                            psQ[:],
                            w_all[:, k, :],
                            xt[:, k, c * 512:(c + 1) * 512],
                            start=(k == 0),
                            stop=(k == KB - 1),
                        )
                    if fin:
                        if scale is None:
                            nc.vector.tensor_copy(
                                dst[:, c * 512:(c + 1) * 512], psQ[:]
                            )
                        else:
                            nc.vector.tensor_scalar_mul(
                                dst[:, c * 512:(c + 1) * 512], psQ[:], scale
                            )

                return [
                    lambda: part(0, 4, False),
                    lambda: part(4, KB, True),
                ]

            qt = {}
            kt = {}

            def qk_work(hp):
                qt[hp] = sb.tile([P, N], F16, tag="qt", bufs=3, name=f"qt{hp}")
                kt[hp] = sb.tile([P, N], F16, tag="kt", bufs=3, name=f"kt{hp}")
                wqt = sb.tile([P, KB, P], F16, tag="wq", bufs=3, name=f"wq{hp}")
                wkt = sb.tile([P, KB, P], F16, tag="wk", bufs=3, name=f"wk{hp}")
                nc.sync.dma_start(wqt[:], wq_d.ap()[hp])
                nc.sync.dma_start(wkt[:], wk_d.ap()[hp])
                out = []
                for c in range(MC):
                    out.extend(qk_proj_parts(hp, c, wqt, qt[hp], 0.125))
                    out.extend(qk_proj_parts(hp, c, wkt, kt[hp], None))
                return out

            def attn_chunk(hp, c, pump_rate=0.5, norm_q=None, prepend_norm=False,
                           fast_norm=False):
                jb_max = min(MB, 4 * c + 4)
                psOa = [
                    ps.tile([P, 512], F32, tag="psO", bufs=2, name="psO0"),
                    ps.tile([P, 512], F32, tag="psO", bufs=2, name="psO1"),
                ]
                pts = {}

                def stage_s(jb):
                    psS = ps.tile([P, 1024], F32, tag="psS", bufs=2, name="psS")
                    r = jb - 4 * c
                    pre = P * r if r > 0 else 0
                    for h2 in range(2):
                        nc.tensor.matmul(
                            psS[:, h2 * 512 + pre:(h2 + 1) * 512],
                            kt[hp][h2 * DK:(h2 + 1) * DK, jb * P:(jb + 1) * P],
                            qt[hp][h2 * DK:(h2 + 1) * DK, c * 512 + pre:(c + 1) * 512],
                            start=True,
                            stop=True,
                            tile_position=(h2 * DK, 0),
                        )
                    pt = sb.tile([P, 1024], F16, tag="pt", bufs=4, name="pt")
                    if pre:
                        # one strided ACT over both heads' valid slices
                        psS3 = psS[:].rearrange("p (h x) -> p h x", h=2)
                        pt3 = pt[:].rearrange("p (h x) -> p h x", h=2)
                        nc.scalar.activation(
                            pt3[:, :, pre:512], psS3[:, :, pre:512], AF.Exp
                        )
                    else:
                        nc.scalar.activation(pt[:], psS[:], AF.Exp)
                    if r >= 0:
                        # causal mask: zero the upper triangle of the diagonal
                        # 128-block (fp16 on SBUF; cheap vs fp32 PSUM add)
                        for h2 in range(2):
                            sl = pt[:, h2 * 512 + pre:h2 * 512 + pre + P]
                            nc.vector.tensor_tensor(
                                sl, sl, tri16[:], mybir.AluOpType.mult
                            )
                    pts[jb] = pt

                def stage_pv(jb):
                    pt = pts.pop(jb)
                    first, last = (jb == 0), (jb == jb_max - 1)
                    r = jb - 4 * c
                    pre = P * r if (r > 0 and not first) else 0
                    for h2 in range(2):
                        h = 2 * hp + h2
                        nc.tensor.matmul(
                            psOa[h2][0:65, pre:512],
                            v[jb][:, h * 65:(h + 1) * 65],
                            pt[:, h2 * 512 + pre:(h2 + 1) * 512],
                            start=first,
                            stop=last,
                            skip_group_check=True,
                        )
                    pump(pump_rate)

                for jb in range(jb_max):
                    stage_s(jb)
                    if jb >= 2:
                        stage_pv(jb - 2)
                stage_pv(jb_max - 2)
                stage_pv(jb_max - 1)

                cpO = [
                    sb.tile([65, 512], F32, tag="sm512", bufs=14, name=f"cpO{h2}")
                    for h2 in range(2)
                ]
                nc.vector.tensor_copy(cpO[0][0:65, :], psOa[0][0:65, :])
                nc.vector.tensor_copy(cpO[1][0:65, :], psOa[1][0:65, :])

                if fast_norm:
                    # tail-critical: broadcast denominators via a K=1 ones
                    # matmul, 64-lane reciprocal, per-head pipelining; filler
                    # matmuls keep the PE clock warm while the DVE/DMA chain
                    # runs so the final out-proj isn't cold-throttled
                    bcD = ps.tile([64, 1024], F32, tag="psS", bufs=2, name="bcD")
                    for h2 in range(2):
                        nc.tensor.matmul(
                            bcD[0:64, h2 * 512:(h2 + 1) * 512],
                            ones32[64:65, 0:64],
                            cpO[h2][64:65, :],
                            start=True,
                            stop=True,
                        )
                    psT = ps.tile([64, 128], F32, tag="psS", bufs=2, name="psT")
                    for i in range(80):
                        nc.tensor.matmul(
                            psT[0:64, 0:128], tri16[:, 0:64], tri16[:],
                            start=(i == 0), stop=(i == 79),
                        )
                    rbcS = sb.tile([64, 1024], F32, tag="sm512", bufs=14, name="rbcS")
                    tmpf = sb.tile([64, 512], F16, tag="sm512", bufs=14, name="tmpf")
                    nc.vector.reciprocal_approx_fast(
                        rbcS[0:64, 0:512], bcD[0:64, 0:512]
                    )
                    nc.vector.tensor_tensor(
                        ot[hp][0:64, c * 512:(c + 1) * 512],
                        cpO[0][0:64, :],
                        rbcS[0:64, 0:512],
                        mybir.AluOpType.mult,
                    )
                    nc.vector.reciprocal_approx_fast(
                        rbcS[0:64, 512:1024], bcD[0:64, 512:1024]
                    )
                    nc.vector.tensor_tensor(
                        tmpf[0:64, :],
                        cpO[1][0:64, :],
                        rbcS[0:64, 512:1024],
                        mybir.AluOpType.mult,
                    )
                    nc.sync.dma_start(
                        ot[hp][64:128, c * 512:(c + 1) * 512], tmpf[0:64, :]
                    )
                    return

                rbc = [
                    sb.tile([64, 512], F32, tag="sm512", bufs=14, name=f"rbc{h2}")
                    for h2 in range(2)
                ]
                tmp1 = sb.tile([64, 512], F16, tag="sm512", bufs=14, name="tmp1")

                nr = sb.tile([1, 1024], F32, tag="nr", bufs=4, name="nr")
                nr2 = sb.tile([1, 1024], F32, tag="nr", bufs=4, name="nr2")

                def norm_piece(stage):
                    if stage == 0:
                        # move denominator rows (lane 64) to lane 0
                        nc.sync.dma_start(nr[0:1, 0:512], cpO[0][64:65, :])
                        nc.sync.dma_start(nr[0:1, 512:1024], cpO[1][64:65, :])
                    elif stage == 1:
                        nc.vector.reciprocal_approx_fast(nr2[0:1, :], nr[0:1, :])
                    elif stage == 2:
                        nc.gpsimd.partition_broadcast(
                            rbc[0][0:64, :], nr2[0:1, 0:512]
                        )
                        nc.gpsimd.partition_broadcast(
                            rbc[1][0:64, :], nr2[0:1, 512:1024]
                        )
                    elif stage == 3:
                        nc.vector.tensor_tensor(
                            ot[hp][0:64, c * 512:(c + 1) * 512],
                            cpO[0][0:64, :],
                            rbc[0][0:64, :],
                            mybir.AluOpType.mult,
                        )
                    elif stage == 4:
                        nc.vector.tensor_tensor(
                            tmp1[0:64, :],
                            cpO[1][0:64, :],
                            rbc[1][0:64, :],
                            mybir.AluOpType.mult,
                        )
                        nc.sync.dma_start(
                            ot[hp][64:128, c * 512:(c + 1) * 512], tmp1[0:64, :]
                        )

                if interleave:
                    if norm_q is None:
                        for st in range(5):
                            dve_deferred.append(lambda st=st: norm_piece(st))
                    elif prepend_norm:
                        stages = [
                            (lambda st=st: norm_piece(st)) for st in range(5)
                        ]
                        norm_q[0:0] = stages[0:2]
                        pos = min(8, len(norm_q))
                        norm_q[pos:pos] = stages[2:5]
                    else:
                        for st in range(5):
                            norm_q.append(lambda st=st: norm_piece(st))
                else:
                    for st in range(5):
                        norm_piece(st)

            # ---- schedule: V proj + QK(hp0) interleaved per xt chunk so the
            # tensor queue never head-of-line blocks on a distant DMA ----
            qk0 = qk_work(0)
            for cc in range(1, MC):
                xt_stream(cc)
            nc.sync.dma_start(
                wo_t[:], wo_d.ap().rearrange("(v p) n -> p v n", p=P)
            )

            def outproj_parts(do, c, tag="proj", split_flush=False):
                cell = {}

                def part(v0, v1, fin):
                    if v0 == 0:
                        cell["ps"] = ps.tile(
                            [P, 512], F32, tag=tag, bufs=2, name="psF"
                        )
                    psF = cell["ps"]
                    for dv in range(v0, v1):
                        nc.tensor.matmul(
                            psF[:],
                            wo_t[:, dv, do * P:(do + 1) * P],
                            ot[dv][:, c * 512:(c + 1) * 512],
                            start=(dv == 0),
                            stop=(dv == NHP - 1),
                        )
                    if fin:
                        o_sb = sb.tile([P, 512], F16, tag="sm512", bufs=14, name="o_sb")
                        halves = ((0, 256), (256, 512)) if split_flush else ((0, 512),)
                        for a, b in halves:
                            nc.vector.tensor_copy(o_sb[:, a:b], psF[:, a:b])
                            nc.sync.dma_start(
                                out_d.ap()[
                                    do * P:(do + 1) * P, c * 512 + a:c * 512 + b
                                ],
                                o_sb[:, a:b],
                            )

                return [lambda: part(0, 2, False), lambda: part(2, NHP, True)]

            for cc in range(MC):
                if cc == 0:
                    # k-part interleave: 8 matmuls can start on the first
                    # half-chunk DMA instead of 4 (avoids FIFO head-of-line)
                    for a, b in [(0, 1), (2, 3)]:
                        cell = {}
                        v_proj_k(a, 0, 4, cell)
                        v_proj_k(b, 0, 4, cell)
                        v_proj_k(a, 4, KB, cell)
                        v_proj_k(b, 4, KB, cell)
                else:
                    for m in range(4 * cc, 4 * cc + 4):
                        v_proj(m)
                for wfn in qk0[4 * cc:4 * cc + 4]:
                    wfn()

            def units_in(chunks):
                return sum(min(MB, 4 * cc + 4) for cc in chunks)

            def once(fn):
                state = {}

                def g():
                    if not state:
                        state["x"] = 1
                        fn()

                return g

            qk3_late = [[], []]
            for t in range(NHP):
                nxt = qk_work(t + 1) if t + 1 < NHP else []
                if interleave and t + 1 == NHP - 1 and nxt:
                    # hold the last head-pair's c2/c3 projection parts back as
                    # pump fodder for its otherwise-dry first two chunks
                    nxt = [once(f) for f in nxt]
                    qk3_late = [nxt[8:12], nxt[12:16]]
                    nxt = nxt[0:8]
                if interleave:
                    deferred.extend(nxt)
                last = t == NHP - 1
                order = list(range(MC))
                for ci, c in enumerate(order):
                    if last and interleave and ci < 2:
                        deferred[0:0] = qk3_late[ci]
                    rem = units_in(order[ci:])
                    hold[0] = 6 if (last and ci < len(order) - 1) else 0
                    rate = min(
                        3.0,
                        max(0, len(deferred) - hold[0]) / max(rem - 8, 1) + 0.3,
                    )
                    attn_chunk(t, c, pump_rate=rate,
                               norm_q=deferred if last else None,
                               fast_norm=last and ci == len(order) - 1)
                    if last and interleave and ci < 2:
                        for fn in qk3_late[ci]:
                            fn()  # no-op if already pumped
                    if last and interleave and ci < len(order) - 1:
                        for do in range(D // P):
                            deferred.extend(outproj_parts(do, c))
                if t < NHP - 1:
                    while dve_deferred:
                        dve_deferred.pop(0)()
                    while deferred:
                        deferred.pop(0)()
                if not interleave:
                    for wfn in nxt:
                        wfn()

            # ---- drain remaining deferred work ----
            while dve_deferred:
                dve_deferred.pop(0)()
            while deferred:
                deferred.pop(0)()
            if interleave:
                # final chunk's out-proj: alternate psum tags (psO banks are
                # free after the fast norm) for a 4-deep psF pipeline
                for do in range(D // P):
                    for th in outproj_parts(
                        do, MC - 1, tag="psO" if do % 2 else "proj",
                        split_flush=(do >= D // P - 2),
                    ):
                        th()
            if not interleave:
                for c in range(MC):
                    for do in range(D // P):
                        for th in outproj_parts(do, c):
                            th()

    nc.compile()
    return nc


def make_core_inputs(X, mask, Wq, Wk, Wv, Wo):
    """Full inputs -> list of 8 per-core input maps (batch-major, head-group minor)."""
    B = X.shape[0]
    # 0/1 keep-mask for the diagonal 128-block: tri16[j, i] = 1 iff query i >= key j
    tri16 = np.ascontiguousarray(
        (np.arange(P)[None, :] >= np.arange(P)[:, None]).astype(np.float16)
    )

    def hp_layout(W, sl):
        # [(k p), (hp m)] -> [hp, p, k, m] contiguous per head-pair block
        return np.ascontiguousarray(
            W[:, sl].astype(np.float16).reshape(KB, P, NHP, P).transpose(2, 1, 0, 3)
        )

    in_maps = []
    for b in range(B):
        xt = np.ascontiguousarray(X[b].T.astype(np.float16))
        for g in range(2):
            sl = slice(g * DH, (g + 1) * DH)
            in_maps.append(
                {
                    "xt": xt,
                    "wq": hp_layout(Wq, sl),
                    "wk": hp_layout(Wk, sl),
                    "wv": np.ascontiguousarray(Wv[:, sl].astype(np.float16)),
                    "wo": np.ascontiguousarray(Wo[sl, :].astype(np.float16)),
                    "tri16": tri16,
                    "ones16": np.ones((P, DK), np.float16),
                }
            )
    return in_maps


def gather_output(results, B=4):
    N = results[0]["outt"].shape[1]
    out = np.empty((B, N, D), np.float32)
    for b in range(B):
        s = (
            results[2 * b]["outt"].astype(np.float32)
            + results[2 * b + 1]["outt"].astype(np.float32)
        )
        out[b] = s.T
    return out


# ---------------------------------------------------------------------------
# Self-contained harness entry: full inputs in, full output out.
# Shards across 8 NeuronCores: core = batch b (4) x head-group g (2 x 8 heads).
# Each core runs a fused flash-style causal MHA for its 8 heads; the host
# sums the two head-group partial outputs per batch (row-parallel W_O).
# ---------------------------------------------------------------------------
from concourse.bass_utils import run_bass_kernel_spmd

_NC_CACHE = {}


def _get_nc():
    if "nc" not in _NC_CACHE:
        _NC_CACHE["nc"] = build(N=2048, interleave=True)
    return _NC_CACHE["nc"]


def kernel(X, mask, Wq, Wk, Wv, Wo):
    X = np.asarray(X, dtype=np.float32)
    mask = np.asarray(mask, dtype=np.float32)
    Wq = np.asarray(Wq, dtype=np.float32)
    Wk = np.asarray(Wk, dtype=np.float32)
    Wv = np.asarray(Wv, dtype=np.float32)
    Wo = np.asarray(Wo, dtype=np.float32)
    in_maps = make_core_inputs(X, mask, Wq, Wk, Wv, Wo)
    nc = _get_nc()
    res = run_bass_kernel_spmd(nc, in_maps, list(range(8)))
    return gather_output(res.results, B=X.shape[0])


# revision 49
# speedup vs baseline: 1.0036x; 1.0036x over previous
"""Fused causal MHA kernel for TRN2, one core = (batch b, head-group g of 8 heads).

Layouts (per core):
  xt_all [128, 8, N]  X[b]^T k-blocks stacked  (k%128 on partitions)
  wv_all [128, 8, 512] column shard            (k%128 on partitions)
  wqt/wkt per hp: [128, 8, 128]
  wo_all [128, 4, 1024] row shard
  tri16 [128, 128] fp16 0/1 lower-triangle keep-mask: tri16[j, i] = (i >= j)
  outt [1024, N] fp16 partial (X attn Wo_g)^T; host sums the two
       head-group partials per batch and transposes.

On-chip:
  qt/kt per head-pair hp: [128, N]; partitions = (h0 d0-63, h1 d0-63).
  v per seq m-block: [128, 8*65]; seq on partitions, 8 heads * (64+ones) on free.
  S^T per (hp, c, jb): psum [128, 1024] = h0|h1; j on partitions, i on free.
  Causal masking is post-exp: pt *= tri16 on the diagonal 128-blocks (fp16 DVE),
  instead of adding -1e9 into PSUM pre-exp (slow fp32 PSUM DVE ops).
  PV col-packed: psumO[0:64] = h0 O^T, [64:128] = h1 O^T; 65th V column of
  ones gives the softmax denominator for free.
"""

import os

# recover cleanly if a previous run left the NeuronCores in a degraded
# power/clock state (observed ~19% slowdowns without this)
os.environ.setdefault("NEURON_RT_RESET_CORES", "1")

import numpy as np
import concourse.bass as bass
import concourse.tile as tile
from concourse import bacc, mybir

F32 = mybir.dt.float32
F16 = mybir.dt.float16
F32R = mybir.dt.float32r
AF = mybir.ActivationFunctionType

P = 128
D = 1024
DH = 512  # head-group width: 8 heads * 64
DK = 64
KB = D // P  # 8 k-blocks
NHP = 4  # head-pairs per core
WARM_MM = 24  # PE warm-up matmuls during the DMA lead-in


def build(N=2048, interleave=True):
    MB = N // P  # seq 128-blocks
    MC = N // 512  # seq 512-chunks
    nc = bacc.Bacc("TRN2", target_bir_lowering=False, debug=False)

    xt_d = nc.dram_tensor("xt", [D, N], F16, kind="ExternalInput")
    # host-pretransposed [hp, p, k, m] so each head-pair's block is contiguous
    wq_d = nc.dram_tensor("wq", [NHP, P, KB, P], F16, kind="ExternalInput")
    wk_d = nc.dram_tensor("wk", [NHP, P, KB, P], F16, kind="ExternalInput")
    wv_d = nc.dram_tensor("wv", [D, DH], F16, kind="ExternalInput")
    wo_d = nc.dram_tensor("wo", [DH, D], F16, kind="ExternalInput")
    tri_d = nc.dram_tensor("tri16", [P, P], F16, kind="ExternalInput")
    ones_d = nc.dram_tensor("ones16", [P, DK], F16, kind="ExternalInput")
    out_d = nc.dram_tensor("outt", [D, N], F16, kind="ExternalOutput")

    with tile.TileContext(nc) as tc:
        with (
            tc.tile_pool(name="sb", bufs=1) as sb,
            tc.tile_pool(name="ps", bufs=1, space="PSUM") as ps,
        ):
            # ---- persistent tiles ----
            ones = sb.tile([P, DK], F16, tag="ones", bufs=1)
            tri16 = sb.tile([P, P], F16, tag="tri", bufs=1)
            wv = sb.tile([P, KB, DH], F16, tag="wv", bufs=1, name="wv_all")
            xt = sb.tile([P, KB, N], F16, tag="xt", bufs=1, name="xt_all")
            v = [sb.tile([P, 8 * 65], F16, tag="v", bufs=MB, name=f"v{m}") for m in range(MB)]
            ot = [sb.tile([P, N], F16, tag="ot", bufs=NHP, name=f"ot{t}") for t in range(NHP)]

            nc.sync.dma_start(ones[:], ones_d.ap())
            nc.sync.dma_start(tri16[:], tri_d.ap())
            wv_src = wv_d.ap().rearrange("(k p) n -> p k n", p=P)
            xt_src = xt_d.ap().rearrange("(k p) n -> p k n", p=P)
            # k-split so the first v_proj matmuls can start on half the data
            nc.sync.dma_start(wv[:, 0:4, :], wv_src[:, 0:4, :])
            nc.sync.dma_start(xt[:, 0:4, 0:512], xt_src[:, 0:4, 0:512])
            nc.sync.dma_start(wv[:, 4:6, :], wv_src[:, 4:6, :])
            nc.sync.dma_start(xt[:, 4:6, 0:512], xt_src[:, 4:6, 0:512])
            nc.sync.dma_start(wv[:, 6:KB, :], wv_src[:, 6:KB, :])
            nc.sync.dma_start(xt[:, 6:KB, 0:512], xt_src[:, 6:KB, 0:512])

            def xt_stream(cc):
                nc.sync.dma_start(
                    xt[:, :, cc * 512:(cc + 1) * 512],
                    xt_src[:, :, cc * 512:(cc + 1) * 512],
                )

            # warm the PE clock gate + the ACT exp table during the DMA lead-in
            warm = sb.tile([P, DK], F16, tag="warm", bufs=1, name="warm")
            nc.scalar.activation(warm[:], ones[:], AF.Exp)
            ones32 = sb.tile([P, DK], F32, tag="ones32", bufs=1)
            nc.gpsimd.memset(ones32[:], 1.0)
            psW = ps.tile([P, 512], F32, tag="proj", bufs=2, name="psW")
            for i in range(WARM_MM):
                nc.tensor.matmul(
                    psW[0:64, 0:128], tri16[:, 0:64], tri16[:],
                    start=(i == 0), stop=(i == WARM_MM - 1),
                )

            wo_t = sb.tile([P, NHP, D], F16, tag="wo", bufs=1, name="wo_all")

            # ---- deferred projection work (pumped between attention units) ----
            deferred = []
            dve_deferred = []
            credit = [0.0]
            hold = [0]

            def pump(rate):
                if dve_deferred:
                    dve_deferred.pop(0)()
                credit[0] += rate
                while credit[0] >= 1.0 and len(deferred) > hold[0]:
                    deferred.pop(0)()
                    credit[0] -= 1.0
                if len(deferred) <= hold[0]:
                    credit[0] = 0.0

            def v_proj_k(m, k0, k1, cell):
                if k0 == 0:
                    cell[m] = ps.tile([P, 512], F32, tag="proj", bufs=2, name="psV")
                psV = cell[m]
                for k in range(k0, k1):
                    nc.tensor.matmul(
                        psV[:],
                        xt[:, k, m * P:(m + 1) * P],
                        wv[:, k, :],
                        start=(k == 0),
                        stop=(k == KB - 1),
                    )
                if k1 == KB:
                    v3 = v[m][:].rearrange("p (h x) -> p h x", x=65)
                    nc.vector.tensor_copy(
                        v3[:, :, 0:64], psV[:].rearrange("p (h x) -> p h x", x=64)
                    )
                    nc.vector.tensor_copy(v3[:, :, 64:65], ones[:, 0:8, None])

            def v_proj(m):
                cell = {}
                v_proj_k(m, 0, KB, cell)

            def qk_proj_parts(hp, c, w_all, dst, scale):
                cell = {}

                def part(k0, k1, fin):
                    if k0 == 0:
                        cell["ps"] = ps.tile(
                            [P, 512], F32, tag="proj", bufs=2, name="psQ"
                        )
                    psQ = cell["ps"]
                    for k in range(k0, k1):
                        nc.tensor.matmul(
                            psQ[:],
                            w_all[:, k, :],
                            xt[:, k, c * 512:(c + 1) * 512],
                            start=(k == 0),
                            stop=(k == KB - 1),
                        )
                    if fin:
                        if scale is None:
                            nc.vector.tensor_copy(
                                dst[:, c * 512:(c + 1) * 512], psQ[:]
                            )
                        else:
                            nc.vector.tensor_scalar_mul(
                                dst[:, c * 512:(c + 1) * 512], psQ[:], scale
                            )

                return [
                    lambda: part(0, 4, False),
                    lambda: part(4, KB, True),
                ]

            qt = {}
            kt = {}

            def qk_work(hp):
                qt[hp] = sb.tile([P, N], F16, tag="qt", bufs=3, name=f"qt{hp}")
                kt[hp] = sb.tile([P, N], F16, tag="kt", bufs=3, name=f"kt{hp}")
                wqt = sb.tile([P, KB, P], F16, tag="wq", bufs=3, name=f"wq{hp}")
                wkt = sb.tile([P, KB, P], F16, tag="wk", bufs=3, name=f"wk{hp}")
                nc.sync.dma_start(wqt[:], wq_d.ap()[hp])
                nc.sync.dma_start(wkt[:], wk_d.ap()[hp])
                out = []
                for c in range(MC):
                    out.extend(qk_proj_parts(hp, c, wqt, qt[hp], 0.125))
                    out.extend(qk_proj_parts(hp, c, wkt, kt[hp], None))
                return out

            def attn_chunk(hp, c, pump_rate=0.5, norm_q=None, prepend_norm=False,
                           fast_norm=False):
                jb_max = min(MB, 4 * c + 4)
                psOa = [
                    ps.tile([P, 512], F32, tag="psO", bufs=2, name="psO0"),
                    ps.tile([P, 512], F32, tag="psO", bufs=2, name="psO1"),
                ]
                pts = {}

                def stage_s(jb):
                    psS = ps.tile([P, 1024], F32, tag="psS", bufs=2, name="psS")
                    r = jb - 4 * c
                    pre = P * r if r > 0 else 0
                    for h2 in range(2):
                        nc.tensor.matmul(
                            psS[:, h2 * 512 + pre:(h2 + 1) * 512],
                            kt[hp][h2 * DK:(h2 + 1) * DK, jb * P:(jb + 1) * P],
                            qt[hp][h2 * DK:(h2 + 1) * DK, c * 512 + pre:(c + 1) * 512],
                            start=True,
                            stop=True,
                            tile_position=(h2 * DK, 0),
                        )
                    pt = sb.tile([P, 1024], F16, tag="pt", bufs=4, name="pt")
                    if pre:
                        # one strided ACT over both heads' valid slices
                        psS3 = psS[:].rearrange("p (h x) -> p h x", h=2)
                        pt3 = pt[:].rearrange("p (h x) -> p h x", h=2)
                        nc.scalar.activation(
                            pt3[:, :, pre:512], psS3[:, :, pre:512], AF.Exp
                        )
                    else:
                        nc.scalar.activation(pt[:], psS[:], AF.Exp)
                    if r >= 0:
                        # causal mask: zero the upper triangle of the diagonal
                        # 128-block (fp16 on SBUF; cheap vs fp32 PSUM add)
                        for h2 in range(2):
                            sl = pt[:, h2 * 512 + pre:h2 * 512 + pre + P]
                            nc.vector.tensor_tensor(
                                sl, sl, tri16[:], mybir.AluOpType.mult
                            )
                    pts[jb] = pt

                def stage_pv(jb):
                    pt = pts.pop(jb)
                    first, last = (jb == 0), (jb == jb_max - 1)
                    r = jb - 4 * c
                    pre = P * r if (r > 0 and not first) else 0
                    for h2 in range(2):
                        h = 2 * hp + h2
                        nc.tensor.matmul(
                            psOa[h2][0:65, pre:512],
                            v[jb][:, h * 65:(h + 1) * 65],
                            pt[:, h2 * 512 + pre:(h2 + 1) * 512],
                            start=first,
                            stop=last,
                            skip_group_check=True,
                        )
                    pump(pump_rate)

                for jb in range(jb_max):
                    stage_s(jb)
                    if jb >= 2:
                        stage_pv(jb - 2)
                stage_pv(jb_max - 2)
                stage_pv(jb_max - 1)

                cpO = [
                    sb.tile([65, 512], F32, tag="sm512", bufs=14, name=f"cpO{h2}")
                    for h2 in range(2)
                ]
                nc.vector.tensor_copy(cpO[0][0:65, :], psOa[0][0:65, :])
                nc.vector.tensor_copy(cpO[1][0:65, :], psOa[1][0:65, :])

                if fast_norm:
                    # tail-critical: broadcast denominators via a K=1 ones
                    # matmul, 64-lane reciprocal, per-head pipelining; filler
                    # matmuls keep the PE clock warm while the DVE/DMA chain
                    # runs so the final out-proj isn't cold-throttled
                    bcD = ps.tile([64, 1024], F32, tag="psS", bufs=2, name="bcD")
                    for h2 in range(2):
                        nc.tensor.matmul(
                            bcD[0:64, h2 * 512:(h2 + 1) * 512],
                            ones32[64:65, 0:64],
                            cpO[h2][64:65, :],
                            start=True,
                            stop=True,
                        )
                    psT = ps.tile([64, 128], F32, tag="psS", bufs=2, name="psT")
                    for i in range(80):
                        nc.tensor.matmul(
                            psT[0:64, 0:128], tri16[:, 0:64], tri16[:],
                            start=(i == 0), stop=(i == 79),
                        )
                    rbcS = sb.tile([64, 1024], F32, tag="sm512", bufs=14, name="rbcS")
                    tmpf = sb.tile([64, 512], F16, tag="sm512", bufs=14, name="tmpf")
                    nc.vector.reciprocal_approx_fast(
                        rbcS[0:64, 0:512], bcD[0:64, 0:512]
                    )
                    nc.vector.tensor_tensor(
                        ot[hp][0:64, c * 512:(c + 1) * 512],
                        cpO[0][0:64, :],
                        rbcS[0:64, 0:512],
                        mybir.AluOpType.mult,
                    )
                    nc.vector.reciprocal_approx_fast(
                        rbcS[0:64, 512:1024], bcD[0:64, 512:1024]
                    )
                    nc.vector.tensor_tensor(
                        tmpf[0:64, :],
                        cpO[1][0:64, :],
                        rbcS[0:64, 512:1024],
                        mybir.AluOpType.mult,
                    )
                    nc.sync.dma_start(
                        ot[hp][64:128, c * 512:(c + 1) * 512], tmpf[0:64, :]
                    )
                    return

                rbc = [
                    sb.tile([64, 512], F32, tag="sm512", bufs=14, name=f"rbc{h2}")
                    for h2 in range(2)
                ]
                tmp1 = sb.tile([64, 512], F16, tag="sm512", bufs=14, name="tmp1")

                nr = sb.tile([1, 1024], F32, tag="nr", bufs=4, name="nr")
                nr2 = sb.tile([1, 1024], F32, tag="nr", bufs=4, name="nr2")

                def norm_piece(stage):
                    if stage == 0:
                        # move denominator rows (lane 64) to lane 0
                        nc.sync.dma_start(nr[0:1, 0:512], cpO[0][64:65, :])
                        nc.sync.dma_start(nr[0:1, 512:1024], cpO[1][64:65, :])
                    elif stage == 1:
                        nc.vector.reciprocal_approx_fast(nr2[0:1, :], nr[0:1, :])
                    elif stage == 2:
                        nc.gpsimd.partition_broadcast(
                            rbc[0][0:64, :], nr2[0:1, 0:512]
                        )
                        nc.gpsimd.partition_broadcast(
                            rbc[1][0:64, :], nr2[0:1, 512:1024]
                        )
                    elif stage == 3:
                        nc.vector.tensor_tensor(
                            ot[hp][0:64, c * 512:(c + 1) * 512],
                            cpO[0][0:64, :],
                            rbc[0][0:64, :],
                            mybir.AluOpType.mult,
                        )
                    elif stage == 4:
                        nc.vector.tensor_tensor(
                            tmp1[0:64, :],
                            cpO[1][0:64, :],
                            rbc[1][0:64, :],
                            mybir.AluOpType.mult,
                        )
                        nc.sync.dma_start(
                            ot[hp][64:128, c * 512:(c + 1) * 512], tmp1[0:64, :]
                        )

                if interleave:
                    if norm_q is None:
                        for st in range(5):
                            dve_deferred.append(lambda st=st: norm_piece(st))
                    elif prepend_norm:
                        stages = [
                            (lambda st=st: norm_piece(st)) for st in range(5)
                        ]
                        norm_q[0:0] = stages[0:2]
                        pos = min(8, len(norm_q))
                        norm_q[pos:pos] = stages[2:5]
                    else:
                        for st in range(5):
                            norm_q.append(lambda st=st: norm_piece(st))
                else:
                    for st in range(5):
                        norm_piece(st)

            # ---- schedule: V proj + QK(hp0) interleaved per xt chunk so the
            # tensor queue never head-of-line blocks on a distant DMA ----
            qk0 = qk_work(0)
            for cc in range(1, MC):
                xt_stream(cc)
            nc.sync.dma_start(
                wo_t[:], wo_d.ap().rearrange("(v p) n -> p v n", p=P)
            )

            def outproj_parts(do, c, tag="proj", split_flush=False):
                cell = {}

                def part(v0, v1, fin):
                    if v0 == 0:
                        cell["ps"] = ps.tile(
                            [P, 512], F32, tag=tag, bufs=2, name="psF"
                        )
                    psF = cell["ps"]
                    for dv in range(v0, v1):
                        nc.tensor.matmul(
                            psF[:],
                            wo_t[:, dv, do * P:(do + 1) * P],
                            ot[dv][:, c * 512:(c + 1) * 512],
                            start=(dv == 0),
                            stop=(dv == NHP - 1),
                        )
                    if fin:
                        o_sb = sb.tile([P, 512], F16, tag="sm512", bufs=14, name="o_sb")
                        halves = ((0, 256), (256, 512)) if split_flush else ((0, 512),)
                        for a, b in halves:
                            nc.vector.tensor_copy(o_sb[:, a:b], psF[:, a:b])
                            nc.sync.dma_start(
                                out_d.ap()[
                                    do * P:(do + 1) * P, c * 512 + a:c * 512 + b
                                ],
                                o_sb[:, a:b],
                            )

                return [lambda: part(0, 2, False), lambda: part(2, NHP, True)]

            for cc in range(MC):
                if cc == 0:
                    # k-part interleave: 8 matmuls can start on the first
                    # half-chunk DMA instead of 4 (avoids FIFO head-of-line)
                    for a, b in [(0, 1), (2, 3)]:
                        cell = {}
                        v_proj_k(a, 0, 4, cell)
                        v_proj_k(b, 0, 4, cell)
                        v_proj_k(a, 4, KB, cell)
                        v_proj_k(b, 4, KB, cell)
                else:
                    for m in range(4 * cc, 4 * cc + 4):
                        v_proj(m)
                for wfn in qk0[4 * cc:4 * cc + 4]:
                    wfn()

            def units_in(chunks):
                return sum(min(MB, 4 * cc + 4) for cc in chunks)

            def once(fn):
                state = {}

                def g():
                    if not state:
                        state["x"] = 1
                        fn()

                return g

            qk3_late = [[], []]
            for t in range(NHP):
                nxt = qk_work(t + 1) if t + 1 < NHP else []
                if interleave and t + 1 == NHP - 1 and nxt:
                    # hold the last head-pair's c2/c3 projection parts back as
                    # pump fodder for its otherwise-dry first two chunks
                    nxt = [once(f) for f in nxt]
                    qk3_late = [nxt[8:12], nxt[12:16]]
                    nxt = nxt[0:8]
                if interleave:
                    deferred.extend(nxt)
                last = t == NHP - 1
                order = list(range(MC))
                for ci, c in enumerate(order):
                    if last and interleave and ci < 2:
                        deferred[0:0] = qk3_late[ci]
                    rem = units_in(order[ci:])
                    hold[0] = 6 if (last and ci < len(order) - 1) else 0
                    rate = min(
                        3.0,
                        max(0, len(deferred) - hold[0]) / max(rem - 8, 1) + 0.3,
                    )
                    attn_chunk(t, c, pump_rate=rate,
                               norm_q=deferred if last else None,
                               fast_norm=last and ci == len(order) - 1)
                    if last and interleave and ci < 2:
                        for fn in qk3_late[ci]:
                            fn()  # no-op if already pumped
                    if last and interleave and ci < len(order) - 1:
                        for do in range(D // P):
                            deferred.extend(outproj_parts(do, c))
                if t < NHP - 1:
                    while dve_deferred:
                        dve_deferred.pop(0)()
                    while deferred:
                        deferred.pop(0)()
                if not interleave:
                    for wfn in nxt:
                        wfn()

            # ---- drain remaining deferred work ----
            while dve_deferred:
                dve_deferred.pop(0)()
            while deferred:
                deferred.pop(0)()
            if interleave:
                # final chunk's out-proj: alternate psum tags (psO banks are
                # free after the fast norm) for a 4-deep psF pipeline
                for do in range(D // P):
                    for th in outproj_parts(
                        do, MC - 1, tag="psO" if do % 2 else "proj",
                        split_flush=False,
                    ):
                        th()
            if not interleave:
                for c in range(MC):
                    for do in range(D // P):
                        for th in outproj_parts(do, c):
                            th()

    nc.compile()
    return nc


def make_core_inputs(X, mask, Wq, Wk, Wv, Wo):
    """Full inputs -> list of 8 per-core input maps (batch-major, head-group minor)."""
    B = X.shape[0]
    # 0/1 keep-mask for the diagonal 128-block: tri16[j, i] = 1 iff query i >= key j
    tri16 = np.ascontiguousarray(
        (np.arange(P)[None, :] >= np.arange(P)[:, None]).astype(np.float16)
    )

    def hp_layout(W, sl):
        # [(k p), (hp m)] -> [hp, p, k, m] contiguous per head-pair block
        return np.ascontiguousarray(
            W[:, sl].astype(np.float16).reshape(KB, P, NHP, P).transpose(2, 1, 0, 3)
        )

    in_maps = []
    for b in range(B):
        xt = np.ascontiguousarray(X[b].T.astype(np.float16))
        for g in range(2):
            sl = slice(g * DH, (g + 1) * DH)
            in_maps.append(
                {
                    "xt": xt,
                    "wq": hp_layout(Wq, sl),
                    "wk": hp_layout(Wk, sl),
                    "wv": np.ascontiguousarray(Wv[:, sl].astype(np.float16)),
                    "wo": np.ascontiguousarray(Wo[sl, :].astype(np.float16)),
                    "tri16": tri16,
                    "ones16": np.ones((P, DK), np.float16),
                }
            )
    return in_maps


def gather_output(results, B=4):
    N = results[0]["outt"].shape[1]
    out = np.empty((B, N, D), np.float32)
    for b in range(B):
        s = (
            results[2 * b]["outt"].astype(np.float32)
            + results[2 * b + 1]["outt"].astype(np.float32)
        )
        out[b] = s.T
    return out


# ---------------------------------------------------------------------------
# Self-contained harness entry: full inputs in, full output out.
# Shards across 8 NeuronCores: core = batch b (4) x head-group g (2 x 8 heads).
# Each core runs a fused flash-style causal MHA for its 8 heads; the host
# sums the two head-group partial outputs per batch (row-parallel W_O).
# ---------------------------------------------------------------------------
from concourse.bass_utils import run_bass_kernel_spmd

_NC_CACHE = {}


def _get_nc():
    if "nc" not in _NC_CACHE:
        _NC_CACHE["nc"] = build(N=2048, interleave=True)
    return _NC_CACHE["nc"]


def kernel(X, mask, Wq, Wk, Wv, Wo):
    X = np.asarray(X, dtype=np.float32)
    mask = np.asarray(mask, dtype=np.float32)
    Wq = np.asarray(Wq, dtype=np.float32)
    Wk = np.asarray(Wk, dtype=np.float32)
    Wv = np.asarray(Wv, dtype=np.float32)
    Wo = np.asarray(Wo, dtype=np.float32)
    in_maps = make_core_inputs(X, mask, Wq, Wk, Wv, Wo)
    nc = _get_nc()
    res = run_bass_kernel_spmd(nc, in_maps, list(range(8)))
    return gather_output(res.results, B=X.shape[0])


# revision 50
# speedup vs baseline: 1.0065x; 1.0029x over previous
"""Fused causal MHA kernel for TRN2, one core = (batch b, head-group g of 8 heads).

Layouts (per core):
  xt_all [128, 8, N]  X[b]^T k-blocks stacked  (k%128 on partitions)
  wv_all [128, 8, 512] column shard            (k%128 on partitions)
  wqt/wkt per hp: [128, 8, 128]
  wo_all [128, 4, 1024] row shard
  tri16 [128, 128] fp16 0/1 lower-triangle keep-mask: tri16[j, i] = (i >= j)
  outt [1024, N] fp16 partial (X attn Wo_g)^T; host sums the two
       head-group partials per batch and transposes.

On-chip:
  qt/kt per head-pair hp: [128, N]; partitions = (h0 d0-63, h1 d0-63).
  v per seq m-block: [128, 8*65]; seq on partitions, 8 heads * (64+ones) on free.
  S^T per (hp, c, jb): psum [128, 1024] = h0|h1; j on partitions, i on free.
  Causal masking is post-exp: pt *= tri16 on the diagonal 128-blocks (fp16 DVE),
  instead of adding -1e9 into PSUM pre-exp (slow fp32 PSUM DVE ops).
  PV col-packed: psumO[0:64] = h0 O^T, [64:128] = h1 O^T; 65th V column of
  ones gives the softmax denominator for free.
"""

import os

# recover cleanly if a previous run left the NeuronCores in a degraded
# power/clock state (observed ~19% slowdowns without this)
os.environ.setdefault("NEURON_RT_RESET_CORES", "1")

import numpy as np
import concourse.bass as bass
import concourse.tile as tile
from concourse import bacc, mybir

F32 = mybir.dt.float32
F16 = mybir.dt.float16
F32R = mybir.dt.float32r
AF = mybir.ActivationFunctionType

P = 128
D = 1024
DH = 512  # head-group width: 8 heads * 64
DK = 64
KB = D // P  # 8 k-blocks
NHP = 4  # head-pairs per core
WARM_MM = 24  # PE warm-up matmuls during the DMA lead-in


def build(N=2048, interleave=True):
    MB = N // P  # seq 128-blocks
    MC = N // 512  # seq 512-chunks
    nc = bacc.Bacc("TRN2", target_bir_lowering=False, debug=False)

    xt_d = nc.dram_tensor("xt", [D, N], F16, kind="ExternalInput")
    # host-pretransposed [hp, p, k, m] so each head-pair's block is contiguous
    wq_d = nc.dram_tensor("wq", [NHP, P, KB, P], F16, kind="ExternalInput")
    wk_d = nc.dram_tensor("wk", [NHP, P, KB, P], F16, kind="ExternalInput")
    wv_d = nc.dram_tensor("wv", [D, DH], F16, kind="ExternalInput")
    wo_d = nc.dram_tensor("wo", [DH, D], F16, kind="ExternalInput")
    tri_d = nc.dram_tensor("tri16", [P, P], F16, kind="ExternalInput")
    ones_d = nc.dram_tensor("ones16", [P, DK], F16, kind="ExternalInput")
    out_d = nc.dram_tensor("outt", [D, N], F16, kind="ExternalOutput")

    with tile.TileContext(nc) as tc:
        with (
            tc.tile_pool(name="sb", bufs=1) as sb,
            tc.tile_pool(name="ps", bufs=1, space="PSUM") as ps,
        ):
            # ---- persistent tiles ----
            ones = sb.tile([P, DK], F16, tag="ones", bufs=1)
            tri16 = sb.tile([P, P], F16, tag="tri", bufs=1)
            wv = sb.tile([P, KB, DH], F16, tag="wv", bufs=1, name="wv_all")
            xt = sb.tile([P, KB, N], F16, tag="xt", bufs=1, name="xt_all")
            v = [sb.tile([P, 8 * 65], F16, tag="v", bufs=MB, name=f"v{m}") for m in range(MB)]
            ot = [sb.tile([P, N], F16, tag="ot", bufs=NHP, name=f"ot{t}") for t in range(NHP)]

            nc.sync.dma_start(ones[:], ones_d.ap())
            nc.sync.dma_start(tri16[:], tri_d.ap())
            wv_src = wv_d.ap().rearrange("(k p) n -> p k n", p=P)
            xt_src = xt_d.ap().rearrange("(k p) n -> p k n", p=P)
            # k-split so the first v_proj matmuls can start on half the data
            nc.sync.dma_start(wv[:, 0:4, :], wv_src[:, 0:4, :])
            nc.sync.dma_start(xt[:, 0:4, 0:512], xt_src[:, 0:4, 0:512])
            nc.sync.dma_start(wv[:, 4:6, :], wv_src[:, 4:6, :])
            nc.sync.dma_start(xt[:, 4:6, 0:512], xt_src[:, 4:6, 0:512])
            nc.sync.dma_start(wv[:, 6:KB, :], wv_src[:, 6:KB, :])
            nc.sync.dma_start(xt[:, 6:KB, 0:512], xt_src[:, 6:KB, 0:512])

            def xt_stream(cc):
                nc.sync.dma_start(
                    xt[:, :, cc * 512:(cc + 1) * 512],
                    xt_src[:, :, cc * 512:(cc + 1) * 512],
                )

            # warm the PE clock gate + the ACT exp table during the DMA lead-in
            warm = sb.tile([P, DK], F16, tag="warm", bufs=1, name="warm")
            nc.scalar.activation(warm[:], ones[:], AF.Exp)
            ones32 = sb.tile([P, DK], F32, tag="ones32", bufs=1)
            nc.gpsimd.memset(ones32[:], 1.0)
            psW = ps.tile([P, 512], F32, tag="proj", bufs=2, name="psW")
            for i in range(WARM_MM):
                nc.tensor.matmul(
                    psW[0:64, 0:128], tri16[:, 0:64], tri16[:],
                    start=(i == 0), stop=(i == WARM_MM - 1),
                )

            wo_t = sb.tile([P, NHP, D], F16, tag="wo", bufs=1, name="wo_all")

            # ---- deferred projection work (pumped between attention units) ----
            deferred = []
            dve_deferred = []
            credit = [0.0]
            hold = [0]

            def pump(rate):
                if dve_deferred:
                    dve_deferred.pop(0)()
                credit[0] += rate
                while credit[0] >= 1.0 and len(deferred) > hold[0]:
                    deferred.pop(0)()
                    credit[0] -= 1.0
                if len(deferred) <= hold[0]:
                    credit[0] = 0.0

            def v_proj_k(m, k0, k1, cell):
                if k0 == 0:
                    cell[m] = ps.tile([P, 512], F32, tag="proj", bufs=2, name="psV")
                psV = cell[m]
                for k in range(k0, k1):
                    nc.tensor.matmul(
                        psV[:],
                        xt[:, k, m * P:(m + 1) * P],
                        wv[:, k, :],
                        start=(k == 0),
                        stop=(k == KB - 1),
                    )
                if k1 == KB:
                    v3 = v[m][:].rearrange("p (h x) -> p h x", x=65)
                    nc.vector.tensor_copy(
                        v3[:, :, 0:64], psV[:].rearrange("p (h x) -> p h x", x=64)
                    )
                    nc.vector.tensor_copy(v3[:, :, 64:65], ones[:, 0:8, None])

            def v_proj(m):
                cell = {}
                v_proj_k(m, 0, KB, cell)

            def qk_proj_parts(hp, c, w_all, dst, scale):
                cell = {}

                def part(k0, k1, fin):
                    if k0 == 0:
                        cell["ps"] = ps.tile(
                            [P, 512], F32, tag="proj", bufs=2, name="psQ"
                        )
                    psQ = cell["ps"]
                    for k in range(k0, k1):
                        nc.tensor.matmul(
                            psQ[:],
                            w_all[:, k, :],
                            xt[:, k, c * 512:(c + 1) * 512],
                            start=(k == 0),
                            stop=(k == KB - 1),
                        )
                    if fin:
                        if scale is None:
                            nc.vector.tensor_copy(
                                dst[:, c * 512:(c + 1) * 512], psQ[:]
                            )
                        else:
                            nc.vector.tensor_scalar_mul(
                                dst[:, c * 512:(c + 1) * 512], psQ[:], scale
                            )

                return [
                    lambda: part(0, 4, False),
                    lambda: part(4, KB, True),
                ]

            qt = {}
            kt = {}

            def qk_work(hp):
                qt[hp] = sb.tile([P, N], F16, tag="qt", bufs=3, name=f"qt{hp}")
                kt[hp] = sb.tile([P, N], F16, tag="kt", bufs=3, name=f"kt{hp}")
                wqt = sb.tile([P, KB, P], F16, tag="wq", bufs=3, name=f"wq{hp}")
                wkt = sb.tile([P, KB, P], F16, tag="wk", bufs=3, name=f"wk{hp}")
                nc.sync.dma_start(wqt[:], wq_d.ap()[hp])
                nc.sync.dma_start(wkt[:], wk_d.ap()[hp])
                out = []
                for c in range(MC):
                    out.extend(qk_proj_parts(hp, c, wqt, qt[hp], 0.125))
                    out.extend(qk_proj_parts(hp, c, wkt, kt[hp], None))
                return out

            def attn_chunk(hp, c, pump_rate=0.5, norm_q=None, prepend_norm=False,
                           fast_norm=False):
                jb_max = min(MB, 4 * c + 4)
                psOa = [
                    ps.tile([P, 512], F32, tag="psO", bufs=2, name="psO0"),
                    ps.tile([P, 512], F32, tag="psO", bufs=2, name="psO1"),
                ]
                pts = {}

                def stage_s(jb):
                    psS = ps.tile([P, 1024], F32, tag="psS", bufs=2, name="psS")
                    r = jb - 4 * c
                    pre = P * r if r > 0 else 0
                    for h2 in range(2):
                        nc.tensor.matmul(
                            psS[:, h2 * 512 + pre:(h2 + 1) * 512],
                            kt[hp][h2 * DK:(h2 + 1) * DK, jb * P:(jb + 1) * P],
                            qt[hp][h2 * DK:(h2 + 1) * DK, c * 512 + pre:(c + 1) * 512],
                            start=True,
                            stop=True,
                            tile_position=(h2 * DK, 0),
                        )
                    pt = sb.tile([P, 1024], F16, tag="pt", bufs=4, name="pt")
                    if pre:
                        # one strided ACT over both heads' valid slices
                        psS3 = psS[:].rearrange("p (h x) -> p h x", h=2)
                        pt3 = pt[:].rearrange("p (h x) -> p h x", h=2)
                        nc.scalar.activation(
                            pt3[:, :, pre:512], psS3[:, :, pre:512], AF.Exp
                        )
                    else:
                        nc.scalar.activation(pt[:], psS[:], AF.Exp)
                    if r >= 0:
                        # causal mask: zero the upper triangle of the diagonal
                        # 128-block (fp16 on SBUF; cheap vs fp32 PSUM add)
                        for h2 in range(2):
                            sl = pt[:, h2 * 512 + pre:h2 * 512 + pre + P]
                            nc.vector.tensor_tensor(
                                sl, sl, tri16[:], mybir.AluOpType.mult
                            )
                    pts[jb] = pt

                def stage_pv(jb):
                    pt = pts.pop(jb)
                    first, last = (jb == 0), (jb == jb_max - 1)
                    r = jb - 4 * c
                    pre = P * r if (r > 0 and not first) else 0
                    for h2 in range(2):
                        h = 2 * hp + h2
                        nc.tensor.matmul(
                            psOa[h2][0:65, pre:512],
                            v[jb][:, h * 65:(h + 1) * 65],
                            pt[:, h2 * 512 + pre:(h2 + 1) * 512],
                            start=first,
                            stop=last,
                            skip_group_check=True,
                        )
                    pump(pump_rate)

                for jb in range(jb_max):
                    stage_s(jb)
                    if jb >= 2:
                        stage_pv(jb - 2)
                stage_pv(jb_max - 2)
                stage_pv(jb_max - 1)

                cpO = [
                    sb.tile([65, 512], F32, tag="sm512", bufs=14, name=f"cpO{h2}")
                    for h2 in range(2)
                ]
                nc.vector.tensor_copy(cpO[0][0:65, :], psOa[0][0:65, :])
                nc.vector.tensor_copy(cpO[1][0:65, :], psOa[1][0:65, :])

                if fast_norm:
                    # tail-critical: broadcast denominators via a K=1 ones
                    # matmul, 64-lane reciprocal, per-head pipelining; filler
                    # matmuls keep the PE clock warm while the DVE/DMA chain
                    # runs so the final out-proj isn't cold-throttled
                    bcD = ps.tile([64, 1024], F32, tag="psS", bufs=2, name="bcD")
                    for h2 in range(2):
                        nc.tensor.matmul(
                            bcD[0:64, h2 * 512:(h2 + 1) * 512],
                            ones32[64:65, 0:64],
                            cpO[h2][64:65, :],
                            start=True,
                            stop=True,
                        )
                    psT = ps.tile([64, 128], F32, tag="psS", bufs=2, name="psT")
                    for i in range(80):
                        nc.tensor.matmul(
                            psT[0:64, 0:128], tri16[:, 0:64], tri16[:],
                            start=(i == 0), stop=(i == 79),
                        )
                    rbcS = sb.tile([64, 1024], F32, tag="sm512", bufs=14, name="rbcS")
                    tmpf = sb.tile([64, 512], F16, tag="sm512", bufs=14, name="tmpf")
                    nc.vector.reciprocal_approx_fast(
                        rbcS[0:64, 0:512], bcD[0:64, 0:512]
                    )
                    nc.vector.tensor_tensor(
                        ot[hp][0:64, c * 512:(c + 1) * 512],
                        cpO[0][0:64, :],
                        rbcS[0:64, 0:512],
                        mybir.AluOpType.mult,
                    )
                    nc.vector.reciprocal_approx_fast(
                        rbcS[0:64, 512:1024], bcD[0:64, 512:1024]
                    )
                    nc.vector.tensor_tensor(
                        tmpf[0:64, :],
                        cpO[1][0:64, :],
                        rbcS[0:64, 512:1024],
                        mybir.AluOpType.mult,
                    )
                    nc.sync.dma_start(
                        ot[hp][64:128, c * 512:(c + 1) * 512], tmpf[0:64, :]
                    )
                    return

                rbc = [
                    sb.tile([64, 512], F32, tag="sm512", bufs=14, name=f"rbc{h2}")
                    for h2 in range(2)
                ]
                tmp1 = sb.tile([64, 512], F16, tag="sm512", bufs=14, name="tmp1")

                nr = sb.tile([1, 1024], F32, tag="nr", bufs=4, name="nr")
                nr2 = sb.tile([1, 1024], F32, tag="nr", bufs=4, name="nr2")

                def norm_piece(stage):
                    if stage == 0:
                        # move denominator rows (lane 64) to lane 0
                        nc.sync.dma_start(nr[0:1, 0:512], cpO[0][64:65, :])
                        nc.sync.dma_start(nr[0:1, 512:1024], cpO[1][64:65, :])
                    elif stage == 1:
                        nc.vector.reciprocal_approx_fast(nr2[0:1, :], nr[0:1, :])
                    elif stage == 2:
                        nc.gpsimd.partition_broadcast(
                            rbc[0][0:64, :], nr2[0:1, 0:512]
                        )
                        nc.gpsimd.partition_broadcast(
                            rbc[1][0:64, :], nr2[0:1, 512:1024]
                        )
                    elif stage == 3:
                        nc.vector.tensor_tensor(
                            ot[hp][0:64, c * 512:(c + 1) * 512],
                            cpO[0][0:64, :],
                            rbc[0][0:64, :],
                            mybir.AluOpType.mult,
                        )
                    elif stage == 4:
                        nc.vector.tensor_tensor(
                            tmp1[0:64, :],
                            cpO[1][0:64, :],
                            rbc[1][0:64, :],
                            mybir.AluOpType.mult,
                        )
                        nc.sync.dma_start(
                            ot[hp][64:128, c * 512:(c + 1) * 512], tmp1[0:64, :]
                        )

                if interleave:
                    if norm_q is None:
                        for st in range(5):
                            dve_deferred.append(lambda st=st: norm_piece(st))
                    elif prepend_norm:
                        stages = [
                            (lambda st=st: norm_piece(st)) for st in range(5)
                        ]
                        norm_q[0:0] = stages[0:2]
                        pos = min(8, len(norm_q))
                        norm_q[pos:pos] = stages[2:5]
                    else:
                        for st in range(5):
                            norm_q.append(lambda st=st: norm_piece(st))
                else:
                    for st in range(5):
                        norm_piece(st)

            # ---- schedule: V proj + QK(hp0) interleaved per xt chunk so the
            # tensor queue never head-of-line blocks on a distant DMA ----
            qk0 = qk_work(0)
            for cc in range(1, MC):
                xt_stream(cc)
            nc.sync.dma_start(
                wo_t[:], wo_d.ap().rearrange("(v p) n -> p v n", p=P)
            )

            def outproj_parts(do, c, tag="proj"):
                cell = {}

                def part(v0, v1, fin):
                    if v0 == 0:
                        cell["ps"] = ps.tile(
                            [P, 512], F32, tag=tag, bufs=2, name="psF"
                        )
                    psF = cell["ps"]
                    for dv in range(v0, v1):
                        nc.tensor.matmul(
                            psF[:],
                            wo_t[:, dv, do * P:(do + 1) * P],
                            ot[dv][:, c * 512:(c + 1) * 512],
                            start=(dv == 0),
                            stop=(dv == NHP - 1),
                        )
                    if fin:
                        o_sb = sb.tile([P, 512], F16, tag="sm512", bufs=14, name="o_sb")
                        nc.vector.tensor_copy(o_sb[:], psF[:])
                        nc.sync.dma_start(
                            out_d.ap()[do * P:(do + 1) * P, c * 512:(c + 1) * 512],
                            o_sb[:],
                        )

                return [lambda: part(0, 2, False), lambda: part(2, NHP, True)]

            for cc in range(MC):
                if cc == 0:
                    # k-part interleave: 8 matmuls can start on the first
                    # half-chunk DMA instead of 4 (avoids FIFO head-of-line)
                    for a, b in [(0, 1), (2, 3)]:
                        cell = {}
                        v_proj_k(a, 0, 4, cell)
                        v_proj_k(b, 0, 4, cell)
                        v_proj_k(a, 4, KB, cell)
                        v_proj_k(b, 4, KB, cell)
                else:
                    for m in range(4 * cc, 4 * cc + 4):
                        v_proj(m)
                for wfn in qk0[4 * cc:4 * cc + 4]:
                    wfn()

            def units_in(chunks):
                return sum(min(MB, 4 * cc + 4) for cc in chunks)

            def once(fn):
                state = {}

                def g():
                    if not state:
                        state["x"] = 1
                        fn()

                return g

            qk3_late = [[], []]
            for t in range(NHP):
                nxt = qk_work(t + 1) if t + 1 < NHP else []
                if interleave and t + 1 == NHP - 1 and nxt:
                    # hold the last head-pair's c2/c3 projection parts back as
                    # pump fodder for its otherwise-dry first two chunks
                    nxt = [once(f) for f in nxt]
                    qk3_late = [nxt[8:12], nxt[12:16]]
                    nxt = nxt[0:8]
                if interleave:
                    deferred.extend(nxt)
                last = t == NHP - 1
                order = list(range(MC))
                for ci, c in enumerate(order):
                    if last and interleave and ci < 2:
                        deferred[0:0] = qk3_late[ci]
                    rem = units_in(order[ci:])
                    hold[0] = 6 if (last and ci < len(order) - 1) else 0
                    rate = min(
                        3.0,
                        max(0, len(deferred) - hold[0]) / max(rem - 8, 1) + 0.3,
                    )
                    attn_chunk(t, c, pump_rate=rate,
                               norm_q=deferred if last else None,
                               fast_norm=last and ci == len(order) - 1)
                    if last and interleave and ci < 2:
                        for fn in qk3_late[ci]:
                            fn()  # no-op if already pumped
                    if last and interleave and ci < len(order) - 1:
                        for do in range(D // P):
                            deferred.extend(outproj_parts(do, c))
                if t < NHP - 1:
                    while dve_deferred:
                        dve_deferred.pop(0)()
                    while deferred:
                        deferred.pop(0)()
                if not interleave:
                    for wfn in nxt:
                        wfn()

            # ---- drain remaining deferred work ----
            while dve_deferred:
                dve_deferred.pop(0)()
            while deferred:
                deferred.pop(0)()
            if interleave:
                # final chunk's out-proj: alternate psum tags (psO banks are
                # free after the fast norm) for a 4-deep psF pipeline
                for do in range(D // P):
                    for th in outproj_parts(
                        do, MC - 1, tag="psO" if do % 2 else "proj"
                    ):
                        th()
            if not interleave:
                for c in range(MC):
                    for do in range(D // P):
                        for th in outproj_parts(do, c):
                            th()

    nc.compile()
    return nc


def make_core_inputs(X, mask, Wq, Wk, Wv, Wo):
    """Full inputs -> list of 8 per-core input maps (batch-major, head-group minor)."""
    B = X.shape[0]
    # 0/1 keep-mask for the diagonal 128-block: tri16[j, i] = 1 iff query i >= key j
    tri16 = np.ascontiguousarray(
        (np.arange(P)[None, :] >= np.arange(P)[:, None]).astype(np.float16)
    )

    def hp_layout(W, sl):
        # [(k p), (hp m)] -> [hp, p, k, m] contiguous per head-pair block
        return np.ascontiguousarray(
            W[:, sl].astype(np.float16).reshape(KB, P, NHP, P).transpose(2, 1, 0, 3)
        )

    in_maps = []
    for b in range(B):
        xt = np.ascontiguousarray(X[b].T.astype(np.float16))
        for g in range(2):
            sl = slice(g * DH, (g + 1) * DH)
            in_maps.append(
                {
                    "xt": xt,
                    "wq": hp_layout(Wq, sl),
                    "wk": hp_layout(Wk, sl),
                    "wv": np.ascontiguousarray(Wv[:, sl].astype(np.float16)),
                    "wo": np.ascontiguousarray(Wo[sl, :].astype(np.float16)),
                    "tri16": tri16,
                    "ones16": np.ones((P, DK), np.float16),
                }
            )
    return in_maps


def gather_output(results, B=4):
    N = results[0]["outt"].shape[1]
    out = np.empty((B, N, D), np.float32)
    for b in range(B):
        s = (
            results[2 * b]["outt"].astype(np.float32)
            + results[2 * b + 1]["outt"].astype(np.float32)
        )
        out[b] = s.T
    return out


# ---------------------------------------------------------------------------
# Self-contained harness entry: full inputs in, full output out.
# Shards across 8 NeuronCores: core = batch b (4) x head-group g (2 x 8 heads).
# Each core runs a fused flash-style causal MHA for its 8 heads; the host
# sums the two head-group partial outputs per batch (row-parallel W_O).
# ---------------------------------------------------------------------------
from concourse.bass_utils import run_bass_kernel_spmd

_NC_CACHE = {}


def _get_nc():
    if "nc" not in _NC_CACHE:
        _NC_CACHE["nc"] = build(N=2048, interleave=True)
    return _NC_CACHE["nc"]


def kernel(X, mask, Wq, Wk, Wv, Wo):
    X = np.asarray(X, dtype=np.float32)
    mask = np.asarray(mask, dtype=np.float32)
    Wq = np.asarray(Wq, dtype=np.float32)
    Wk = np.asarray(Wk, dtype=np.float32)
    Wv = np.asarray(Wv, dtype=np.float32)
    Wo = np.asarray(Wo, dtype=np.float32)
    in_maps = make_core_inputs(X, mask, Wq, Wk, Wv, Wo)
    nc = _get_nc()
    res = run_bass_kernel_spmd(nc, in_maps, list(range(8)))
    return gather_output(res.results, B=X.shape[0])


# revision 51
# speedup vs baseline: 1.0087x; 1.0022x over previous
"""Fused causal MHA kernel for TRN2, one core = (batch b, head-group g of 8 heads).

Layouts (per core):
  xt_all [128, 8, N]  X[b]^T k-blocks stacked  (k%128 on partitions)
  wv_all [128, 8, 512] column shard            (k%128 on partitions)
  wqt/wkt per hp: [128, 8, 128]
  wo_all [128, 4, 1024] row shard
  tri16 [128, 128] fp16 0/1 lower-triangle keep-mask: tri16[j, i] = (i >= j)
  outt [1024, N] fp16 partial (X attn Wo_g)^T; host sums the two
       head-group partials per batch and transposes.

On-chip:
  qt/kt per head-pair hp: [128, N]; partitions = (h0 d0-63, h1 d0-63).
  v per seq m-block: [128, 8*65]; seq on partitions, 8 heads * (64+ones) on free.
  S^T per (hp, c, jb): psum [128, 1024] = h0|h1; j on partitions, i on free.
  Causal masking is post-exp: pt *= tri16 on the diagonal 128-blocks (fp16 DVE),
  instead of adding -1e9 into PSUM pre-exp (slow fp32 PSUM DVE ops).
  PV col-packed: psumO[0:64] = h0 O^T, [64:128] = h1 O^T; 65th V column of
  ones gives the softmax denominator for free.
"""

import os

# recover cleanly if a previous run left the NeuronCores in a degraded
# power/clock state (observed ~19% slowdowns without this)
os.environ.setdefault("NEURON_RT_RESET_CORES", "1")

import numpy as np
import concourse.bass as bass
import concourse.tile as tile
from concourse import bacc, mybir

F32 = mybir.dt.float32
F16 = mybir.dt.float16
F32R = mybir.dt.float32r
AF = mybir.ActivationFunctionType

P = 128
D = 1024
DH = 512  # head-group width: 8 heads * 64
DK = 64
KB = D // P  # 8 k-blocks
NHP = 4  # head-pairs per core
WARM_MM = 24  # PE warm-up matmuls during the DMA lead-in


def build(N=2048, interleave=True):
    MB = N // P  # seq 128-blocks
    MC = N // 512  # seq 512-chunks
    nc = bacc.Bacc("TRN2", target_bir_lowering=False, debug=False)

    xt_d = nc.dram_tensor("xt", [D, N], F16, kind="ExternalInput")
    # host-pretransposed [hp, p, k, m] so each head-pair's block is contiguous
    wq_d = nc.dram_tensor("wq", [NHP, P, KB, P], F16, kind="ExternalInput")
    wk_d = nc.dram_tensor("wk", [NHP, P, KB, P], F16, kind="ExternalInput")
    wv_d = nc.dram_tensor("wv", [D, DH], F16, kind="ExternalInput")
    wo_d = nc.dram_tensor("wo", [DH, D], F16, kind="ExternalInput")
    tri_d = nc.dram_tensor("tri16", [P, P], F16, kind="ExternalInput")
    ones_d = nc.dram_tensor("ones16", [P, DK], F16, kind="ExternalInput")
    out_d = nc.dram_tensor("outt", [D, N], F16, kind="ExternalOutput")

    with tile.TileContext(nc) as tc:
        with (
            tc.tile_pool(name="sb", bufs=1) as sb,
            tc.tile_pool(name="ps", bufs=1, space="PSUM") as ps,
        ):
            # ---- persistent tiles ----
            ones = sb.tile([P, DK], F16, tag="ones", bufs=1)
            tri16 = sb.tile([P, P], F16, tag="tri", bufs=1)
            wv = sb.tile([P, KB, DH], F16, tag="wv", bufs=1, name="wv_all")
            xt = sb.tile([P, KB, N], F16, tag="xt", bufs=1, name="xt_all")
            v = [sb.tile([P, 8 * 65], F16, tag="v", bufs=MB, name=f"v{m}") for m in range(MB)]
            ot = [sb.tile([P, N], F16, tag="ot", bufs=NHP, name=f"ot{t}") for t in range(NHP)]

            nc.sync.dma_start(ones[:], ones_d.ap())
            nc.sync.dma_start(tri16[:], tri_d.ap())
            wv_src = wv_d.ap().rearrange("(k p) n -> p k n", p=P)
            xt_src = xt_d.ap().rearrange("(k p) n -> p k n", p=P)
            # k-split so the first v_proj matmuls can start on half the data
            nc.sync.dma_start(wv[:, 0:4, :], wv_src[:, 0:4, :])
            nc.sync.dma_start(xt[:, 0:4, 0:512], xt_src[:, 0:4, 0:512])
            nc.sync.dma_start(wv[:, 4:6, :], wv_src[:, 4:6, :])
            nc.sync.dma_start(xt[:, 4:6, 0:512], xt_src[:, 4:6, 0:512])
            nc.sync.dma_start(wv[:, 6:KB, :], wv_src[:, 6:KB, :])
            nc.sync.dma_start(xt[:, 6:KB, 0:512], xt_src[:, 6:KB, 0:512])

            def xt_stream(cc):
                nc.sync.dma_start(
                    xt[:, :, cc * 512:(cc + 1) * 512],
                    xt_src[:, :, cc * 512:(cc + 1) * 512],
                )

            # warm the PE clock gate + the ACT exp table during the DMA lead-in
            warm = sb.tile([P, DK], F16, tag="warm", bufs=1, name="warm")
            nc.scalar.activation(warm[:], ones[:], AF.Exp)
            ones32 = sb.tile([P, DK], F32, tag="ones32", bufs=1)
            nc.gpsimd.memset(ones32[:], 1.0)
            psW = ps.tile([P, 512], F32, tag="proj", bufs=2, name="psW")
            for i in range(WARM_MM):
                nc.tensor.matmul(
                    psW[0:64, 0:128], tri16[:, 0:64], tri16[:],
                    start=(i == 0), stop=(i == WARM_MM - 1),
                )

            wo_t = sb.tile([P, NHP, D], F16, tag="wo", bufs=1, name="wo_all")

            # ---- deferred projection work (pumped between attention units) ----
            deferred = []
            dve_deferred = []
            credit = [0.0]
            hold = [0]

            def pump(rate):
                if dve_deferred:
                    dve_deferred.pop(0)()
                credit[0] += rate
                while credit[0] >= 1.0 and len(deferred) > hold[0]:
                    deferred.pop(0)()
                    credit[0] -= 1.0
                if len(deferred) <= hold[0]:
                    credit[0] = 0.0

            def v_proj_k(m, k0, k1, cell):
                if k0 == 0:
                    cell[m] = ps.tile([P, 512], F32, tag="proj", bufs=2, name="psV")
                psV = cell[m]
                for k in range(k0, k1):
                    nc.tensor.matmul(
                        psV[:],
                        xt[:, k, m * P:(m + 1) * P],
                        wv[:, k, :],
                        start=(k == 0),
                        stop=(k == KB - 1),
                    )
                if k1 == KB:
                    v3 = v[m][:].rearrange("p (h x) -> p h x", x=65)
                    nc.vector.tensor_copy(
                        v3[:, :, 0:64], psV[:].rearrange("p (h x) -> p h x", x=64)
                    )
                    nc.vector.tensor_copy(v3[:, :, 64:65], ones[:, 0:8, None])

            def v_proj(m):
                cell = {}
                v_proj_k(m, 0, KB, cell)

            def qk_proj_parts(hp, c, w_all, dst, scale):
                cell = {}

                def part(k0, k1, fin):
                    if k0 == 0:
                        cell["ps"] = ps.tile(
                            [P, 512], F32, tag="proj", bufs=2, name="psQ"
                        )
                    psQ = cell["ps"]
                    for k in range(k0, k1):
                        nc.tensor.matmul(
                            psQ[:],
                            w_all[:, k, :],
                            xt[:, k, c * 512:(c + 1) * 512],
                            start=(k == 0),
                            stop=(k == KB - 1),
                        )
                    if fin:
                        if scale is None:
                            nc.vector.tensor_copy(
                                dst[:, c * 512:(c + 1) * 512], psQ[:]
                            )
                        else:
                            nc.vector.tensor_scalar_mul(
                                dst[:, c * 512:(c + 1) * 512], psQ[:], scale
                            )

                return [
                    lambda: part(0, 4, False),
                    lambda: part(4, KB, True),
                ]

            qt = {}
            kt = {}

            def qk_work(hp):
                qt[hp] = sb.tile([P, N], F16, tag="qt", bufs=3, name=f"qt{hp}")
                kt[hp] = sb.tile([P, N], F16, tag="kt", bufs=3, name=f"kt{hp}")
                wqt = sb.tile([P, KB, P], F16, tag="wq", bufs=3, name=f"wq{hp}")
                wkt = sb.tile([P, KB, P], F16, tag="wk", bufs=3, name=f"wk{hp}")
                nc.sync.dma_start(wqt[:], wq_d.ap()[hp])
                nc.sync.dma_start(wkt[:], wk_d.ap()[hp])
                out = []
                for c in range(MC):
                    out.extend(qk_proj_parts(hp, c, wqt, qt[hp], 0.125))
                    out.extend(qk_proj_parts(hp, c, wkt, kt[hp], None))
                return out

            def attn_chunk(hp, c, pump_rate=0.5, norm_q=None, prepend_norm=False,
                           fast_norm=False):
                jb_max = min(MB, 4 * c + 4)
                psOa = [
                    ps.tile([P, 512], F32, tag="psO", bufs=2, name="psO0"),
                    ps.tile([P, 512], F32, tag="psO", bufs=2, name="psO1"),
                ]
                pts = {}

                def stage_s(jb):
                    psS = ps.tile([P, 1024], F32, tag="psS", bufs=2, name="psS")
                    r = jb - 4 * c
                    pre = P * r if r > 0 else 0
                    for h2 in range(2):
                        nc.tensor.matmul(
                            psS[:, h2 * 512 + pre:(h2 + 1) * 512],
                            kt[hp][h2 * DK:(h2 + 1) * DK, jb * P:(jb + 1) * P],
                            qt[hp][h2 * DK:(h2 + 1) * DK, c * 512 + pre:(c + 1) * 512],
                            start=True,
                            stop=True,
                            tile_position=(h2 * DK, 0),
                        )
                    pt = sb.tile([P, 1024], F16, tag="pt", bufs=4, name="pt")
                    if pre:
                        # one strided ACT over both heads' valid slices
                        psS3 = psS[:].rearrange("p (h x) -> p h x", h=2)
                        pt3 = pt[:].rearrange("p (h x) -> p h x", h=2)
                        nc.scalar.activation(
                            pt3[:, :, pre:512], psS3[:, :, pre:512], AF.Exp
                        )
                    else:
                        nc.scalar.activation(pt[:], psS[:], AF.Exp)
                    if r >= 0:
                        # causal mask: zero the upper triangle of the diagonal
                        # 128-block (fp16 on SBUF; cheap vs fp32 PSUM add)
                        for h2 in range(2):
                            sl = pt[:, h2 * 512 + pre:h2 * 512 + pre + P]
                            nc.vector.tensor_tensor(
                                sl, sl, tri16[:], mybir.AluOpType.mult
                            )
                    pts[jb] = pt

                def stage_pv(jb):
                    pt = pts.pop(jb)
                    first, last = (jb == 0), (jb == jb_max - 1)
                    r = jb - 4 * c
                    pre = P * r if (r > 0 and not first) else 0
                    for h2 in range(2):
                        h = 2 * hp + h2
                        nc.tensor.matmul(
                            psOa[h2][0:65, pre:512],
                            v[jb][:, h * 65:(h + 1) * 65],
                            pt[:, h2 * 512 + pre:(h2 + 1) * 512],
                            start=first,
                            stop=last,
                            skip_group_check=True,
                        )
                    pump(pump_rate)

                for jb in range(jb_max):
                    stage_s(jb)
                    if jb >= 2:
                        stage_pv(jb - 2)
                stage_pv(jb_max - 2)
                stage_pv(jb_max - 1)

                cpO = [
                    sb.tile([65, 512], F32, tag="sm512", bufs=14, name=f"cpO{h2}")
                    for h2 in range(2)
                ]
                nc.vector.tensor_copy(cpO[0][0:65, :], psOa[0][0:65, :])
                nc.vector.tensor_copy(cpO[1][0:65, :], psOa[1][0:65, :])

                if fast_norm:
                    # tail-critical: broadcast denominators via a K=1 ones
                    # matmul, 64-lane reciprocal, per-head pipelining; filler
                    # matmuls keep the PE clock warm while the DVE/DMA chain
                    # runs so the final out-proj isn't cold-throttled
                    bcD = ps.tile([64, 1024], F32, tag="psS", bufs=2, name="bcD")
                    for h2 in range(2):
                        nc.tensor.matmul(
                            bcD[0:64, h2 * 512:(h2 + 1) * 512],
                            ones32[64:65, 0:64],
                            cpO[h2][64:65, :],
                            start=True,
                            stop=True,
                        )
                    psT = ps.tile([64, 128], F32, tag="psS", bufs=2, name="psT")
                    for i in range(80):
                        nc.tensor.matmul(
                            psT[0:64, 0:128], tri16[:, 0:64], tri16[:],
                            start=(i == 0), stop=(i == 79),
                        )
                    rbcS = sb.tile([64, 1024], F32, tag="sm512", bufs=14, name="rbcS")
                    tmpf = sb.tile([64, 512], F16, tag="sm512", bufs=14, name="tmpf")
                    nc.vector.reciprocal_approx_fast(
                        rbcS[0:64, 0:512], bcD[0:64, 0:512]
                    )
                    nc.vector.tensor_tensor(
                        ot[hp][0:64, c * 512:(c + 1) * 512],
                        cpO[0][0:64, :],
                        rbcS[0:64, 0:512],
                        mybir.AluOpType.mult,
                    )
                    nc.vector.reciprocal_approx_fast(
                        rbcS[0:64, 512:1024], bcD[0:64, 512:1024]
                    )
                    nc.vector.tensor_tensor(
                        tmpf[0:64, :],
                        cpO[1][0:64, :],
                        rbcS[0:64, 512:1024],
                        mybir.AluOpType.mult,
                    )
                    nc.sync.dma_start(
                        ot[hp][64:128, c * 512:(c + 1) * 512], tmpf[0:64, :]
                    )
                    return

                rbc = [
                    sb.tile([64, 512], F32, tag="sm512", bufs=14, name=f"rbc{h2}")
                    for h2 in range(2)
                ]
                tmp1 = sb.tile([64, 512], F16, tag="sm512", bufs=14, name="tmp1")

                nr = sb.tile([1, 1024], F32, tag="nr", bufs=4, name="nr")
                nr2 = sb.tile([1, 1024], F32, tag="nr", bufs=4, name="nr2")

                def norm_piece(stage):
                    if stage == 0:
                        # move denominator rows (lane 64) to lane 0
                        nc.sync.dma_start(nr[0:1, 0:512], cpO[0][64:65, :])
                        nc.sync.dma_start(nr[0:1, 512:1024], cpO[1][64:65, :])
                    elif stage == 1:
                        nc.vector.reciprocal_approx_fast(nr2[0:1, :], nr[0:1, :])
                    elif stage == 2:
                        nc.gpsimd.partition_broadcast(
                            rbc[0][0:64, :], nr2[0:1, 0:512]
                        )
                        nc.gpsimd.partition_broadcast(
                            rbc[1][0:64, :], nr2[0:1, 512:1024]
                        )
                    elif stage == 3:
                        nc.vector.tensor_tensor(
                            ot[hp][0:64, c * 512:(c + 1) * 512],
                            cpO[0][0:64, :],
                            rbc[0][0:64, :],
                            mybir.AluOpType.mult,
                        )
                    elif stage == 4:
                        nc.vector.tensor_tensor(
                            tmp1[0:64, :],
                            cpO[1][0:64, :],
                            rbc[1][0:64, :],
                            mybir.AluOpType.mult,
                        )
                        nc.sync.dma_start(
                            ot[hp][64:128, c * 512:(c + 1) * 512], tmp1[0:64, :]
                        )

                if interleave:
                    if norm_q is None:
                        for st in range(5):
                            dve_deferred.append(lambda st=st: norm_piece(st))
                    elif prepend_norm:
                        stages = [
                            (lambda st=st: norm_piece(st)) for st in range(5)
                        ]
                        norm_q[0:0] = stages[0:2]
                        pos = min(8, len(norm_q))
                        norm_q[pos:pos] = stages[2:5]
                    else:
                        for st in range(5):
                            norm_q.append(lambda st=st: norm_piece(st))
                else:
                    for st in range(5):
                        norm_piece(st)

            # ---- schedule: V proj + QK(hp0) interleaved per xt chunk so the
            # tensor queue never head-of-line blocks on a distant DMA ----
            qk0 = qk_work(0)
            for cc in range(1, MC):
                xt_stream(cc)
            nc.sync.dma_start(
                wo_t[:], wo_d.ap().rearrange("(v p) n -> p v n", p=P)
            )

            def outproj_parts(do, c, tag="proj"):
                cell = {}

                def part(v0, v1, fin):
                    if v0 == 0:
                        cell["ps"] = ps.tile(
                            [P, 512], F32, tag=tag, bufs=2, name="psF"
                        )
                    psF = cell["ps"]
                    for dv in range(v0, v1):
                        nc.tensor.matmul(
                            psF[:],
                            wo_t[:, dv, do * P:(do + 1) * P],
                            ot[dv][:, c * 512:(c + 1) * 512],
                            start=(dv == 0),
                            stop=(dv == NHP - 1),
                        )
                    if fin:
                        o_sb = sb.tile([P, 512], F16, tag="sm512", bufs=14, name="o_sb")
                        nc.vector.tensor_copy(o_sb[:], psF[:])
                        # tail path: alternate DMA issue across the two HWDGE
                        # queues (sync + scalar) so the final flush pipelines
                        eng = nc.scalar if tag == "psO" else nc.sync
                        eng.dma_start(
                            out_d.ap()[do * P:(do + 1) * P, c * 512:(c + 1) * 512],
                            o_sb[:],
                        )

                return [lambda: part(0, 2, False), lambda: part(2, NHP, True)]

            for cc in range(MC):
                if cc == 0:
                    # k-part interleave: 8 matmuls can start on the first
                    # half-chunk DMA instead of 4 (avoids FIFO head-of-line)
                    for a, b in [(0, 1), (2, 3)]:
                        cell = {}
                        v_proj_k(a, 0, 4, cell)
                        v_proj_k(b, 0, 4, cell)
                        v_proj_k(a, 4, KB, cell)
                        v_proj_k(b, 4, KB, cell)
                else:
                    for m in range(4 * cc, 4 * cc + 4):
                        v_proj(m)
                for wfn in qk0[4 * cc:4 * cc + 4]:
                    wfn()

            def units_in(chunks):
                return sum(min(MB, 4 * cc + 4) for cc in chunks)

            def once(fn):
                state = {}

                def g():
                    if not state:
                        state["x"] = 1
                        fn()

                return g

            qk3_late = [[], []]
            for t in range(NHP):
                nxt = qk_work(t + 1) if t + 1 < NHP else []
                if interleave and t + 1 == NHP - 1 and nxt:
                    # hold the last head-pair's c2/c3 projection parts back as
                    # pump fodder for its otherwise-dry first two chunks
                    nxt = [once(f) for f in nxt]
                    qk3_late = [nxt[8:12], nxt[12:16]]
                    nxt = nxt[0:8]
                if interleave:
                    deferred.extend(nxt)
                last = t == NHP - 1
                order = list(range(MC))
                for ci, c in enumerate(order):
                    if last and interleave and ci < 2:
                        deferred[0:0] = qk3_late[ci]
                    rem = units_in(order[ci:])
                    hold[0] = 6 if (last and ci < len(order) - 1) else 0
                    rate = min(
                        3.0,
                        max(0, len(deferred) - hold[0]) / max(rem - 8, 1) + 0.3,
                    )
                    attn_chunk(t, c, pump_rate=rate,
                               norm_q=deferred if last else None,
                               fast_norm=last and ci == len(order) - 1)
                    if last and interleave and ci < 2:
                        for fn in qk3_late[ci]:
                            fn()  # no-op if already pumped
                    if last and interleave and ci < len(order) - 1:
                        for do in range(D // P):
                            deferred.extend(outproj_parts(do, c))
                if t < NHP - 1:
                    while dve_deferred:
                        dve_deferred.pop(0)()
                    while deferred:
                        deferred.pop(0)()
                if not interleave:
                    for wfn in nxt:
                        wfn()

            # ---- drain remaining deferred work ----
            while dve_deferred:
                dve_deferred.pop(0)()
            while deferred:
                deferred.pop(0)()
            if interleave:
                # final chunk's out-proj: alternate psum tags (psO banks are
                # free after the fast norm) for a 4-deep psF pipeline
                for do in range(D // P):
                    for th in outproj_parts(
                        do, MC - 1, tag="psO" if do % 2 else "proj"
                    ):
                        th()
            if not interleave:
                for c in range(MC):
                    for do in range(D // P):
                        for th in outproj_parts(do, c):
                            th()

    nc.compile()
    return nc


def make_core_inputs(X, mask, Wq, Wk, Wv, Wo):
    """Full inputs -> list of 8 per-core input maps (batch-major, head-group minor)."""
    B = X.shape[0]
    # 0/1 keep-mask for the diagonal 128-block: tri16[j, i] = 1 iff query i >= key j
    tri16 = np.ascontiguousarray(
        (np.arange(P)[None, :] >= np.arange(P)[:, None]).astype(np.float16)
    )

    def hp_layout(W, sl):
        # [(k p), (hp m)] -> [hp, p, k, m] contiguous per head-pair block
        return np.ascontiguousarray(
            W[:, sl].astype(np.float16).reshape(KB, P, NHP, P).transpose(2, 1, 0, 3)
        )

    in_maps = []
    for b in range(B):
        xt = np.ascontiguousarray(X[b].T.astype(np.float16))
        for g in range(2):
            sl = slice(g * DH, (g + 1) * DH)
            in_maps.append(
                {
                    "xt": xt,
                    "wq": hp_layout(Wq, sl),
                    "wk": hp_layout(Wk, sl),
                    "wv": np.ascontiguousarray(Wv[:, sl].astype(np.float16)),
                    "wo": np.ascontiguousarray(Wo[sl, :].astype(np.float16)),
                    "tri16": tri16,
                    "ones16": np.ones((P, DK), np.float16),
                }
            )
    return in_maps


def gather_output(results, B=4):
    N = results[0]["outt"].shape[1]
    out = np.empty((B, N, D), np.float32)
    for b in range(B):
        s = (
            results[2 * b]["outt"].astype(np.float32)
            + results[2 * b + 1]["outt"].astype(np.float32)
        )
        out[b] = s.T
    return out


# ---------------------------------------------------------------------------
# Self-contained harness entry: full inputs in, full output out.
# Shards across 8 NeuronCores: core = batch b (4) x head-group g (2 x 8 heads).
# Each core runs a fused flash-style causal MHA for its 8 heads; the host
# sums the two head-group partial outputs per batch (row-parallel W_O).
# ---------------------------------------------------------------------------
from concourse.bass_utils import run_bass_kernel_spmd

_NC_CACHE = {}


def _get_nc():
    if "nc" not in _NC_CACHE:
        _NC_CACHE["nc"] = build(N=2048, interleave=True)
    return _NC_CACHE["nc"]


def kernel(X, mask, Wq, Wk, Wv, Wo):
    X = np.asarray(X, dtype=np.float32)
    mask = np.asarray(mask, dtype=np.float32)
    Wq = np.asarray(Wq, dtype=np.float32)
    Wk = np.asarray(Wk, dtype=np.float32)
    Wv = np.asarray(Wv, dtype=np.float32)
    Wo = np.asarray(Wo, dtype=np.float32)
    in_maps = make_core_inputs(X, mask, Wq, Wk, Wv, Wo)
    nc = _get_nc()
    res = run_bass_kernel_spmd(nc, in_maps, list(range(8)))
    return gather_output(res.results, B=X.shape[0])
